# revision 54
# baseline (speedup 1.0000x reference)
"""MAE self-attention (sparse_attention) Trainium2 Bass kernel, v3.

Sharding: 8 cores = batch(2) x head-groups(4 groups of 3 heads).

Structure (see git history for the evolution):
  - The embx key (key 0 of 2049) is handled on the HOST as a rank-1
    correction, so the device sees exactly 2048 keys = 16 aligned tiles
    (no padded tile; the no-attend diagonal lands on block diagonals).
  - kv projection (bf16): k^T via W-stationary matmuls with head-packed
    m-tiles [k_h0|k_h1] and [k_h2|k_h2]; v via xT-stationary matmuls.
  - scores^T[j, q]: row-packed matmul pairs on PE quadrant rows 0-63 /
    64-127.  Heads h0/h1 pair with EACH OTHER (same query chunk, two psum
    halves), so no kT or q duplication is needed; h2 pairs with itself
    via the [k_h2|k_h2] projection layout (only q_h2 ships duplicated).
  - p = exp(scale*scores + keybias): mostly ACT (Exp activation, masked
    keys underflow to exactly 0); a tunable subset of tiles runs on DVE
    via a bf16 Schraudolph exp (one tensor_scalar to int16 bits; masked
    keys get multiplier 0 -> +0.0).  Diagonal zeroed by [128,128] bf16
    mask multiplies on DVE.
  - pv TRANSPOSED: out[q, d] accumulates in PSUM [128q, 4, 65] (four
    128-query chains share one bank) over the 16 key tiles with pt
    stationary; column 64 is the ones-column -> softmax denominator.
  - PE warm-up spin amortizes the tensor engine's DVFS ramp; inputs
    arrive as a few large DMAs ordered by first use (the DMA transfer
    stage is a serial resource).
Host divides by the denominator after adding the embx rank-1 term.
"""

import ml_dtypes
import numpy as np

import concourse.bacc as bacc
import concourse.bass as bass  # noqa: F401
import concourse.mybir as mybir
import concourse.tile as tile
from concourse.bass_utils import run_bass_kernel_spmd

F32 = mybir.dt.float32
BF16 = mybir.dt.bfloat16
I16 = mybir.dt.int16
Exp = mybir.ActivationFunctionType.Exp
AluMult = mybir.AluOpType.mult
AluAdd = mybir.AluOpType.add

B = 2
S = 2048          # queries; also device-side keys (hidden states only)
HID = 768
H = 12
D = 64
G = 3             # heads per core
NCORE = 8
NT = 16           # key tiles of 128
KC = HID // 128   # 6 contraction chunks
NEG = -10000.0
SCALE = 0.125     # D ** -0.5

# Schraudolph bf16 exp: exp(y) ~= bitcast_bf16(int16(y*SA16 + SB16)).
# SA16 = 128/ln2; SB16 tuned numerically (rms rel err ~1.8%, max ~4.3%;
# within 0.25 of optimal for either round or trunc float->int converts).
SA16 = 184.66496414152556
SB16 = 16248.75
# key-tiles per block whose exp runs on DVE instead of ACT
OFFLOAD_T = (2, 5, 8, 11, 14)

WCOLS = 448       # W layout: [k_h0|k_h1 | k_h2|k_h2 | v_h0 v_h1 v_h2 (192)]
LOOKAHEAD = 2


def _build_nc(reps=1):
    nc = bacc.Bacc(None, target_bir_lowering=False)

    # pre-chunked host layouts: partition dim first so each input needs
    # only a few large DMAs (the transfer stage is serial; ~900ns fixed
    # semaphore-propagation cost per transfer)
    xT_d = nc.dram_tensor("xT", [128, KC, S], BF16, kind="ExternalInput")
    w_d = nc.dram_tensor("W", [128, KC, WCOLS], BF16, kind="ExternalInput")
    # qT slot 0 = [q_h0 ; q_h1], slot 1 = [q_h2 ; q_h2]
    qT_d = nc.dram_tensor("qT", [128, 2, S], BF16, kind="ExternalInput")
    # ct = [bk(2) | bv(192) | kb(16) | s1(16) | s2(16)]
    ct_d = nc.dram_tensor("ct", [128, 242], F32, kind="ExternalInput")
    dm_d = nc.dram_tensor("dm", [128, 128], BF16, kind="ExternalInput")
    out_d = nc.dram_tensor("outT", [G, 2, 2, 128, 4 * (D + 1)], F32,
                           kind="ExternalOutput")

    with tile.TileContext(nc) as tc:
        with (
            tc.tile_pool(name="const", bufs=1) as cpool,
            tc.tile_pool(name="pt", bufs=2) as ptpool,
            tc.tile_pool(name="ovec", bufs=3) as opool,
            tc.tile_pool(name="psS", bufs=3, space="PSUM") as pss,
            tc.tile_pool(name="psV", bufs=2, space="PSUM") as psv,
        ):
            xT_sb = cpool.tile([128, KC, S], BF16)
            w_sb = cpool.tile([128, KC, WCOLS], BF16)
            qT_sb = cpool.tile([128, 2, S], BF16)
            kTa_sb = cpool.tile([128, S], BF16)    # [k_h0 ; k_h1]
            kTc_sb = cpool.tile([128, S], BF16)    # [k_h2 ; k_h2]
            v_sb = cpool.tile([128, NT, G, D + 1], BF16)
            ct_sb = cpool.tile([128, 242], F32)
            dm_sb = cpool.tile([128, 128], BF16)
            bk_sb = ct_sb[:, 0:2]
            bv_sb = ct_sb[:, 2:194]
            kb_sb = ct_sb[:, 194:210]
            s1_sb = ct_sb[:, 210:226]
            s2_sb = ct_sb[:, 226:242]

            # PE warm-up: throwaway matmuls so the tensor engine's DVFS
            # ramp (slow p-states for the first ~3us of activity) is spent
            # before the first real projection chain arrives.
            wu_sb = cpool.tile([128, 512], BF16)
            wups = pss.tile([128, 1024], F32, tag="ps", name="wups")
            nc.vector.memset(wu_sb, 0.0)
            for i in range(16):
                nc.tensor.matmul(
                    wups[:, 0:512], wu_sb[:, 0:128], wu_sb,
                    start=True, stop=True,
                )

            # --- input DMAs, all on the SP ring, ordered by first use (the
            # transfer stage is a serial resource, so a second ring buys no
            # bandwidth — and triggers on the ACT ring would hold the ACT
            # sequencer ~700ns each in front of the first exps) ---
            nc.sync.dma_start(out=w_sb, in_=w_d[:, :, :])
            nc.sync.dma_start(out=xT_sb[:, :, 0:512], in_=xT_d[:, :, 0:512])
            nc.sync.dma_start(out=ct_sb, in_=ct_d[:, :])
            nc.sync.dma_start(out=qT_sb[:, 0, :], in_=qT_d[:, 0, :])
            nc.sync.dma_start(out=xT_sb[:, :, 512:1024],
                              in_=xT_d[:, :, 512:1024])
            nc.sync.dma_start(out=dm_sb, in_=dm_d[:, :])
            nc.sync.dma_start(out=xT_sb[:, :, 1024:1536],
                              in_=xT_d[:, :, 1024:1536])
            nc.sync.dma_start(out=xT_sb[:, :, 1536:2048],
                              in_=xT_d[:, :, 1536:2048])
            nc.sync.dma_start(out=qT_sb[:, 1, :], in_=qT_d[:, 1, :])

            for rep in range(reps):
                # ---- kv projection ----
                def proj_k_chain(ct, ncol):
                    ps = pss.tile([128, 1024], F32, tag="ps")
                    c0 = ncol * 512
                    for kc in range(KC):
                        nc.tensor.matmul(
                            ps[:, 0:512],
                            w_sb[:, kc, ct * 128:(ct + 1) * 128],
                            xT_sb[:, kc, c0:c0 + 512],
                            start=(kc == 0),
                            stop=(kc == KC - 1),
                        )
                    dst = kTa_sb if ct == 0 else kTc_sb
                    nc.vector.tensor_scalar_add(
                        dst[:, c0:c0 + 512], ps[:, 0:512],
                        bk_sb[:, ct:ct + 1],
                    )

                def proj_v_tile(t):
                    ps = pss.tile([128, 1024], F32, tag="ps")
                    for kc in range(KC):
                        nc.tensor.matmul(
                            ps[:, 0:192],
                            xT_sb[:, kc, t * 128:(t + 1) * 128],
                            w_sb[:, kc, 2 * 128:2 * 128 + 192],
                            start=(kc == 0),
                            stop=(kc == KC - 1),
                        )
                    nc.vector.tensor_add(
                        v_sb[:, t, :, 0:D],
                        ps[:, 0:G * D].rearrange("p (h d) -> p h d", h=G),
                        bv_sb.rearrange("p (h d) -> p h d", h=G),
                    )
                    nc.vector.memset(v_sb[:, t, :, D:D + 1], 1.0)

                # ---- attention ----
                # blocks: (pair, qc).  pair 0 = heads h0/h1, query chunk
                # qc*512; pair 1 = h2 self-paired, chunks 2qc / 2qc+1 on the
                # two psum halves.
                blocks = [(0, qc) for qc in range(4)] + [(1, j) for j in (0, 1)]
                pt_tiles = {}

                def emit_scores(pair, qc, t):
                    ps = pss.tile([128, 1024], F32, tag="ps")
                    kT = kTa_sb if pair == 0 else kTc_sb
                    if pair == 0:
                        qA = qT_sb[0:64, 0, qc * 512:(qc + 1) * 512]
                        qB = qT_sb[64:128, 0, qc * 512:(qc + 1) * 512]
                    else:
                        qA = qT_sb[0:64, 1, qc * 1024:qc * 1024 + 512]
                        qB = qT_sb[64:128, 1, qc * 1024 + 512:(qc + 1) * 1024]
                    nc.tensor.matmul(
                        ps[:, 0:512], kT[0:64, t * 128:(t + 1) * 128], qA,
                        start=True, stop=True, tile_position=(0, 0),
                    )
                    nc.tensor.matmul(
                        ps[:, 512:1024], kT[64:128, t * 128:(t + 1) * 128], qB,
                        start=True, stop=True, tile_position=(64, 0),
                    )
                    return ps

                def emit_exp(pair, qc, t, ps, extra=False):
                    pt = ptpool.tile([128, 1024], BF16, tag=f"pt{t}")
                    if t in OFFLOAD_T or extra:
                        nc.vector.tensor_scalar(
                            pt.bitcast(I16), ps,
                            s1_sb[:, t:t + 1], s2_sb[:, t:t + 1],
                            AluMult, AluAdd,
                        )
                    else:
                        nc.scalar.activation(
                            pt, ps, Exp, bias=kb_sb[:, t:t + 1], scale=SCALE
                        )
                    # zero the q == key block diagonal
                    c = (t % 4) * 128
                    if pair == 0:
                        if t // 4 == qc:
                            nc.vector.tensor_mul(
                                pt[:, c:c + 128], pt[:, c:c + 128], dm_sb)
                            nc.vector.tensor_mul(
                                pt[:, 512 + c:512 + c + 128],
                                pt[:, 512 + c:512 + c + 128], dm_sb)
                    else:
                        if t // 4 == 2 * qc:
                            nc.vector.tensor_mul(
                                pt[:, c:c + 128], pt[:, c:c + 128], dm_sb)
                        elif t // 4 == 2 * qc + 1:
                            nc.vector.tensor_mul(
                                pt[:, 512 + c:512 + c + 128],
                                pt[:, 512 + c:512 + c + 128], dm_sb)
                    pt_tiles[(pair, qc, t)] = pt

                # pv chains: chain (h, qt) covers queries qt*128..+128 of
                # head h.  Four consecutive chains of one head share a
                # 1-bank PSUM tile and one output DMA.
                pv_cur = [None]

                def pt_col(h, qt, t):
                    if h < 2:
                        key = (0, qt // 4, t)
                        col = 512 * h + (qt % 4) * 128
                    else:
                        key = (1, qt // 8, t)
                        col = 512 * ((qt % 8) // 4) + (qt % 4) * 128
                    return pt_tiles[key][:, col:col + 128]

                def emit_pv_chain(h, qt, t0=0, t1=NT, pv=None):
                    if pv is None:
                        if qt % 4 == 0 and t0 == 0:
                            pv_cur[0] = psv.tile(
                                [128, 4, D + 1], F32, tag="pv",
                                name=f"pv_{rep}_{h}_{qt}")
                        pv = pv_cur[0]
                    for t in range(t0, t1):
                        nc.tensor.matmul(
                            pv[:, qt % 4, :],
                            pt_col(h, qt, t),
                            v_sb[:, t, h, :],
                            start=(t == 0),
                            stop=(t == NT - 1),
                        )
                    if t1 < NT:
                        return pv
                    if qt % 4 == 3:
                        ov = opool.tile([128, 4, D + 1], F32, tag="ov",
                                        name=f"ov_{rep}_{h}_{qt}")
                        nc.vector.tensor_copy(ov, pv)
                        nc.sync.dma_start(
                            out=out_d[h, qt // 8, (qt // 4) % 2, :, :],
                            in_=ov.rearrange("p a b -> p (a b)"),
                        )

                # chains of block bi, in emission order (groups of 4)
                def block_chains(bi):
                    pair, qc = blocks[bi]
                    if pair == 0:
                        return ([(0, 4 * qc + i) for i in range(4)]
                                + [(1, 4 * qc + i) for i in range(4)])
                    return [(2, 8 * qc + i) for i in range(8)]

                # Remaining projection work rides inside the attention step
                # stream, timed to the xT column-slice DMA arrivals.
                vq = list(range(NT))
                bwork = {0: {}, 1: {}, 2: {}}
                for st in range(NT):
                    if st == 2:
                        bwork[0][st] = lambda: proj_k_chain(0, 1)
                    elif st == 6:
                        bwork[0][st] = lambda: proj_k_chain(0, 2)
                    elif st == 10:
                        bwork[0][st] = lambda: proj_k_chain(0, 3)
                    elif st == 14:
                        pass
                    else:
                        bwork[0][st] = (
                            lambda tt: (lambda: proj_v_tile(tt)))(vq.pop(0))
                for st in (0, 2, 4, 6):
                    bwork[1][st] = (
                        lambda tt: (lambda: proj_v_tile(tt)))(vq.pop(0))
                for i, st in enumerate((0, 2, 4, 6)):
                    bwork[2][st] = (
                        lambda n: (lambda: proj_k_chain(1, n)))(i)

                proj_k_chain(0, 0)

                # flat software pipeline over all (block, t) steps: scores
                # run LOOKAHEAD steps ahead of exp, across block boundaries
                steps = [(bi, t) for bi in range(len(blocks))
                         for t in range(NT)]
                n_steps = len(steps)
                final = len(blocks) - 1
                prev_ps = {}
                pvqs = {bi: block_chains(bi - 1)
                        for bi in range(1, len(blocks))}
                for i in range(n_steps + LOOKAHEAD):
                    if i < n_steps:
                        bi, t = steps[i]
                        pair, qc = blocks[bi]
                        prev_ps[(bi, t)] = emit_scores(pair, qc, t)
                        if t == 0 and bi >= 2:
                            # leftover pv chains of earlier blocks
                            for b in range(1, bi):
                                while pvqs.get(b):
                                    emit_pv_chain(*pvqs[b].pop(0))
                        if bi in bwork and t in bwork[bi]:
                            bwork[bi][t]()
                        elif t % 2 == 1 and t >= (7 if bi == 1 else 3):
                            # pv chains of the previous block on odd steps
                            # (cross-block lookahead exps land first; block
                            # 0's chains also wait for the last v-tile)
                            if pvqs.get(bi):
                                emit_pv_chain(*pvqs[bi].pop(0))
                        if bi == final and t == 14:
                            while pvqs[final]:
                                emit_pv_chain(*pvqs[final].pop(0))
                    j = i - LOOKAHEAD
                    if j >= 0:
                        bj, tj = steps[j]
                        pj, qj = blocks[bj]
                        # the final block's last exps go to DVE so the chain
                        # drain isn't serialized behind the ACT queue
                        extra = bj == final and tj >= NT - 3
                        emit_exp(pj, qj, tj, prev_ps.pop((bj, tj)), extra)
                for ci in range(8):
                    emit_pv_chain(*block_chains(final)[ci])

    nc.finalize()
    return nc


_NC = None


def _get_nc():
    global _NC
    if _NC is None:
        _NC = _build_nc()
    return _NC


def _host_prep(hidden_states, embx, expanded_embx, Wkv_w, Wkv_b,
               attention_mask, mlm_mask):
    hs = np.asarray(hidden_states, np.float32)
    qx = np.asarray(expanded_embx, np.float32)
    w = np.asarray(Wkv_w, np.float32)
    bb = np.asarray(Wkv_b, np.float32)
    am = np.asarray(attention_mask).astype(bool)
    mm = np.asarray(mlm_mask).astype(bool)

    valid = (am & ~mm).astype(np.float32)              # (B, S)

    dm = np.ones((128, 128), ml_dtypes.bfloat16)
    np.fill_diagonal(dm, 0.0)

    # per-batch tensors; xT pre-chunked as [128, KC, S]
    xT = [np.ascontiguousarray(
              hs[b].T.astype(ml_dtypes.bfloat16)
              .reshape(KC, 128, S).transpose(1, 0, 2))
          for b in range(B)]
    kbf, s1f, s2f = [], [], []
    for b in range(B):
        v = valid[b]                                   # (S,)
        kb = np.where(v > 0, 0.0, NEG).astype(np.float32)
        s1 = (v * (SA16 * SCALE)).astype(np.float32)
        s2 = (v * SB16).astype(np.float32)
        kbf.append(kb.reshape(NT, 128).T)
        s1f.append(s1.reshape(NT, 128).T)
        s2f.append(s2.reshape(NT, 128).T)

    # per-group weight layouts, pre-chunked as [128, KC, WCOLS]
    wg_l, bk_l, bv_l = [], [], []
    for g in range(4):
        k_cols = slice(192 * g, 192 * g + 192)
        v_cols = slice(768 + 192 * g, 768 + 192 * g + 192)
        wk = w[:, k_cols]                              # (768, 192)
        parts = [wk[:, 0:128],                         # [k_h0 | k_h1]
                 wk[:, 128:192], wk[:, 128:192],       # [k_h2 | k_h2]
                 w[:, v_cols]]                         # v (192)
        wg = np.concatenate(parts, axis=1).astype(ml_dtypes.bfloat16)
        wg_l.append(np.ascontiguousarray(
            wg.reshape(KC, 128, WCOLS).transpose(1, 0, 2)))
        bkk = bb[k_cols]
        bk = np.stack([bkk[0:128],
                       np.concatenate([bkk[128:192], bkk[128:192]])], axis=1)
        bk_l.append(bk.astype(np.float32))
        bv_l.append(np.broadcast_to(
            bb[v_cols], (128, G * D)).astype(np.float32))

    in_maps = []
    for c in range(NCORE):
        b, g = divmod(c, 4)
        ct = np.concatenate(
            [bk_l[g], bv_l[g], kbf[b], s1f[b], s2f[b]], axis=1)
        qg = qx[b][:, 192 * g:192 * g + 192].T         # (192, S)
        qt = np.empty((128, 2, S), ml_dtypes.bfloat16)
        qt[0:64, 0, :] = qg[0:64].astype(ml_dtypes.bfloat16)
        qt[64:128, 0, :] = qg[64:128].astype(ml_dtypes.bfloat16)
        qt[0:64, 1, :] = qg[128:192].astype(ml_dtypes.bfloat16)
        qt[64:128, 1, :] = qt[0:64, 1, :]
        in_maps.append(dict(
            xT=xT[b], W=wg_l[g], qT=np.ascontiguousarray(qt),
            ct=np.ascontiguousarray(ct), dm=dm,
        ))
    return in_maps


def _host_post(results, embx, expanded_embx, Wkv_w, Wkv_b):
    ex = np.asarray(embx, np.float32)                  # (B, 1, HID)
    qx = np.asarray(expanded_embx, np.float32)
    w = np.asarray(Wkv_w, np.float32)
    bb = np.asarray(Wkv_b, np.float32)

    # embx key: k/v projections + per-query weights, on host
    kv_eb = ex[:, 0, :] @ w + bb                       # (B, 2*HID)
    k_eb = kv_eb[:, :HID].reshape(B, H, D)
    v_eb = kv_eb[:, HID:].reshape(B, H, D)
    q3 = qx.reshape(B, S, H, D)
    s_eb = np.einsum("bshd,bhd->bsh", q3, k_eb)        # (B, S, H)
    p_eb = np.exp(SCALE * s_eb.astype(np.float64)).astype(np.float32)

    out = np.empty((B, S, HID), np.float32)
    for c in range(NCORE):
        b, g = divmod(c, 4)
        # [G, 2, 2, 128, 4, 65] -> (h, half, group, slot, row) -> (G, S, 65)
        ot = (results[c]["outT"]
              .reshape(G, 2, 2, 128, 4, D + 1)
              .transpose(0, 1, 2, 4, 3, 5)
              .reshape(G, S, D + 1))
        for h in range(G):
            hh = 3 * g + h
            num = ot[h, :, :D] + p_eb[b, :, hh:hh + 1] * v_eb[b, hh]
            den = ot[h, :, D] + p_eb[b, :, hh]
            out[b, :, 192 * g + 64 * h:192 * g + 64 * h + 64] = (
                num / den[:, None]
            )
    return out


def kernel(hidden_states, embx, expanded_embx, Wkv_w, Wkv_b,
           attention_mask, mlm_mask):
    in_maps = _host_prep(hidden_states, embx, expanded_embx, Wkv_w, Wkv_b,
                         attention_mask, mlm_mask)
    nc = _get_nc()
    res = run_bass_kernel_spmd(nc, in_maps, list(range(NCORE)))
    return _host_post(res.results, embx, expanded_embx, Wkv_w, Wkv_b)


# revision 60
# speedup vs baseline: 6.1312x; 6.1312x over previous
"""MAE self-attention (sparse_attention) Trainium2 Bass kernel, v3.

Sharding: 8 cores = batch(2) x head-groups(4 groups of 3 heads).

Structure (see git history for the evolution):
  - The embx key (key 0 of 2049) is handled on the HOST as a rank-1
    correction, so the device sees exactly 2048 keys = 16 aligned tiles
    (no padded tile; the no-attend diagonal lands on block diagonals).
  - kv projection (bf16): k^T via W-stationary matmuls with head-packed
    m-tiles [k_h0|k_h1] and [k_h2|k_h2]; v via xT-stationary matmuls.
  - scores^T[j, q]: row-packed matmul pairs on PE quadrant rows 0-63 /
    64-127.  Heads h0/h1 pair with EACH OTHER (same query chunk, two psum
    halves), so no kT or q duplication is needed; h2 pairs with itself
    via the [k_h2|k_h2] projection layout (only q_h2 ships duplicated).
  - p = exp(scale*scores + keybias): mostly ACT (Exp activation, masked
    keys underflow to exactly 0); a tunable subset of tiles runs on DVE
    via a bf16 Schraudolph exp (one tensor_scalar to int16 bits; masked
    keys get multiplier 0 -> +0.0).  Diagonal zeroed by [128,128] bf16
    mask multiplies on DVE.
  - pv TRANSPOSED: out[q, d] accumulates in PSUM [128q, 4, 65] (four
    128-query chains share one bank) over the 16 key tiles with pt
    stationary; column 64 is the ones-column -> softmax denominator.
  - PE warm-up spin amortizes the tensor engine's DVFS ramp; inputs
    arrive as a few large DMAs ordered by first use (the DMA transfer
    stage is a serial resource).
Host divides by the denominator after adding the embx rank-1 term.
"""

import ml_dtypes
import numpy as np

import concourse.bacc as bacc
import concourse.bass as bass  # noqa: F401
import concourse.mybir as mybir
import concourse.tile as tile
from concourse.bass_utils import run_bass_kernel_spmd

F32 = mybir.dt.float32
BF16 = mybir.dt.bfloat16
I16 = mybir.dt.int16
Exp = mybir.ActivationFunctionType.Exp
AluMult = mybir.AluOpType.mult
AluAdd = mybir.AluOpType.add

B = 2
S = 2048          # queries; also device-side keys (hidden states only)
HID = 768
H = 12
D = 64
G = 3             # heads per core
NCORE = 8
NT = 16           # key tiles of 128
KC = HID // 128   # 6 contraction chunks
NEG = -10000.0
SCALE = 0.125     # D ** -0.5

# Schraudolph bf16 exp: exp(y) ~= bitcast_bf16(int16(y*SA16 + SB16)).
# SA16 = 128/ln2; SB16 tuned numerically (rms rel err ~1.8%, max ~4.3%;
# within 0.25 of optimal for either round or trunc float->int converts).
SA16 = 184.66496414152556
SB16 = 16248.75
# key-tiles per block whose exp runs on DVE instead of ACT
OFFLOAD_T = (2, 5, 8, 11, 14)

WCOLS = 448       # W layout: [k_h0|k_h1 | k_h2|k_h2 | v_h0 v_h1 v_h2 (192)]
LOOKAHEAD = 2


def _build_nc(reps=1):
    nc = bacc.Bacc(None, target_bir_lowering=False)

    # pre-chunked host layouts: partition dim first so each input needs
    # only a few large DMAs (the transfer stage is serial; ~900ns fixed
    # semaphore-propagation cost per transfer)
    xT_d = nc.dram_tensor("xT", [128, KC, S], BF16, kind="ExternalInput")
    w_d = nc.dram_tensor("W", [128, KC, WCOLS], BF16, kind="ExternalInput")
    # qT slot 0 = [q_h0 ; q_h1], slot 1 = [q_h2 ; q_h2]
    qT_d = nc.dram_tensor("qT", [128, 2, S], BF16, kind="ExternalInput")
    # ct = [bk(2) | bv(192) | kb(16) | s1(16) | s2(16)]
    ct_d = nc.dram_tensor("ct", [128, 242], F32, kind="ExternalInput")
    dm_d = nc.dram_tensor("dm", [128, 128], BF16, kind="ExternalInput")
    out_d = nc.dram_tensor("outT", [G, 2, 2, 128, 4 * (D + 1)], F32,
                           kind="ExternalOutput")

    with tile.TileContext(nc) as tc:
        with (
            tc.tile_pool(name="const", bufs=1) as cpool,
            tc.tile_pool(name="pt", bufs=3) as ptpool,
            tc.tile_pool(name="ovec", bufs=3) as opool,
            tc.tile_pool(name="psS", bufs=3, space="PSUM") as pss,
            tc.tile_pool(name="psV", bufs=2, space="PSUM") as psv,
        ):
            xT_sb = cpool.tile([128, KC, S], BF16)
            w_sb = cpool.tile([128, KC, WCOLS], BF16)
            qT_sb = cpool.tile([128, 2, S], BF16)
            kTa_sb = cpool.tile([128, S], BF16)    # [k_h0 ; k_h1]
            kTc_sb = cpool.tile([128, S], BF16)    # [k_h2 ; k_h2]
            v_sb = cpool.tile([128, NT, G, D + 1], BF16)
            ct_sb = cpool.tile([128, 242], F32)
            dm_sb = cpool.tile([128, 128], BF16)
            bk_sb = ct_sb[:, 0:2]
            bv_sb = ct_sb[:, 2:194]
            kb_sb = ct_sb[:, 194:210]
            s1_sb = ct_sb[:, 210:226]
            s2_sb = ct_sb[:, 226:242]

            # PE warm-up: throwaway matmuls so the tensor engine's DVFS
            # ramp (slow p-states for the first ~3us of activity) is spent
            # before the first real projection chain arrives.
            wu_sb = cpool.tile([128, 512], BF16)
            wups = pss.tile([128, 1024], F32, tag="ps", name="wups")
            nc.gpsimd.memset(wu_sb, 0.0)
            for i in range(16):
                nc.tensor.matmul(
                    wups[:, 0:512], wu_sb[:, 0:128], wu_sb,
                    start=True, stop=True,
                )

            # --- input DMAs, all on the SP ring, ordered by first use (the
            # transfer stage is a serial resource, so a second ring buys no
            # bandwidth — and triggers on the ACT ring would hold the ACT
            # sequencer ~700ns each in front of the first exps) ---
            nc.sync.dma_start(out=w_sb, in_=w_d[:, :, :])
            nc.sync.dma_start(out=xT_sb[:, :, 0:512], in_=xT_d[:, :, 0:512])
            nc.sync.dma_start(out=ct_sb, in_=ct_d[:, :])
            nc.sync.dma_start(out=qT_sb[:, 0, :], in_=qT_d[:, 0, :])
            nc.sync.dma_start(out=xT_sb[:, :, 512:1024],
                              in_=xT_d[:, :, 512:1024])
            nc.sync.dma_start(out=dm_sb, in_=dm_d[:, :])
            nc.sync.dma_start(out=xT_sb[:, :, 1024:1536],
                              in_=xT_d[:, :, 1024:1536])
            nc.sync.dma_start(out=xT_sb[:, :, 1536:2048],
                              in_=xT_d[:, :, 1536:2048])
            nc.sync.dma_start(out=qT_sb[:, 1, :], in_=qT_d[:, 1, :])

            for rep in range(reps):
                # ---- kv projection ----
                def proj_k_chain(ct, ncol):
                    ps = pss.tile([128, 1024], F32, tag="ps")
                    c0 = ncol * 512
                    for kc in range(KC):
                        nc.tensor.matmul(
                            ps[:, 0:512],
                            w_sb[:, kc, ct * 128:(ct + 1) * 128],
                            xT_sb[:, kc, c0:c0 + 512],
                            start=(kc == 0),
                            stop=(kc == KC - 1),
                        )
                    dst = kTa_sb if ct == 0 else kTc_sb
                    # bias-add on ACT (idle during the projection phase)
                    nc.scalar.add(dst[:, c0:c0 + 512], ps[:, 0:512],
                                  bk_sb[:, ct:ct + 1])

                def proj_v_tile(t):
                    ps = pss.tile([128, 1024], F32, tag="ps")
                    for kc in range(KC):
                        nc.tensor.matmul(
                            ps[:, 0:192],
                            xT_sb[:, kc, t * 128:(t + 1) * 128],
                            w_sb[:, kc, 2 * 128:2 * 128 + 192],
                            start=(kc == 0),
                            stop=(kc == KC - 1),
                        )
                    nc.vector.tensor_add(
                        v_sb[:, t, :, 0:D],
                        ps[:, 0:G * D].rearrange("p (h d) -> p h d", h=G),
                        bv_sb.rearrange("p (h d) -> p h d", h=G),
                    )
                    nc.vector.memset(v_sb[:, t, :, D:D + 1], 1.0)

                # ---- attention ----
                # blocks: (pair, qc).  pair 0 = heads h0/h1, query chunk
                # qc*512; pair 1 = h2 self-paired, chunks 2qc / 2qc+1 on the
                # two psum halves.
                blocks = [(0, qc) for qc in range(4)] + [(1, j) for j in (0, 1)]
                pt_tiles = {}

                def emit_scores(pair, qc, t):
                    ps = pss.tile([128, 1024], F32, tag="ps")
                    kT = kTa_sb if pair == 0 else kTc_sb
                    if pair == 0:
                        qA = qT_sb[0:64, 0, qc * 512:(qc + 1) * 512]
                        qB = qT_sb[64:128, 0, qc * 512:(qc + 1) * 512]
                    else:
                        qA = qT_sb[0:64, 1, qc * 1024:qc * 1024 + 512]
                        qB = qT_sb[64:128, 1, qc * 1024 + 512:(qc + 1) * 1024]
                    nc.tensor.matmul(
                        ps[:, 0:512], kT[0:64, t * 128:(t + 1) * 128], qA,
                        start=True, stop=True, tile_position=(0, 0),
                    )
                    nc.tensor.matmul(
                        ps[:, 512:1024], kT[64:128, t * 128:(t + 1) * 128], qB,
                        start=True, stop=True, tile_position=(64, 0),
                    )
                    return ps

                def emit_exp(pair, qc, t, ps, extra=False):
                    pt = ptpool.tile([128, 1024], BF16, tag=f"pt{t}")
                    if t in OFFLOAD_T or extra:
                        nc.vector.tensor_scalar(
                            pt.bitcast(I16), ps,
                            s1_sb[:, t:t + 1], s2_sb[:, t:t + 1],
                            AluMult, AluAdd,
                        )
                    else:
                        nc.scalar.activation(
                            pt, ps, Exp, bias=kb_sb[:, t:t + 1], scale=SCALE
                        )
                    # zero the q == key block diagonal (on the otherwise-idle
                    # GPSIMD engine; pt lives in SBUF which Pool can access)
                    c = (t % 4) * 128
                    if pair == 0:
                        if t // 4 == qc:
                            nc.gpsimd.tensor_mul(
                                pt[:, c:c + 128], pt[:, c:c + 128], dm_sb)
                            nc.gpsimd.tensor_mul(
                                pt[:, 512 + c:512 + c + 128],
                                pt[:, 512 + c:512 + c + 128], dm_sb)
                    else:
                        if t // 4 == 2 * qc:
                            nc.gpsimd.tensor_mul(
                                pt[:, c:c + 128], pt[:, c:c + 128], dm_sb)
                        elif t // 4 == 2 * qc + 1:
                            nc.gpsimd.tensor_mul(
                                pt[:, 512 + c:512 + c + 128],
                                pt[:, 512 + c:512 + c + 128], dm_sb)
                    pt_tiles[(pair, qc, t)] = pt

                # pv chains: chain (h, qt) covers queries qt*128..+128 of
                # head h.  Four consecutive chains of one head share a
                # 1-bank PSUM tile and one output DMA.
                pv_cur = [None]

                def pt_col(h, qt, t):
                    if h < 2:
                        key = (0, qt // 4, t)
                        col = 512 * h + (qt % 4) * 128
                    else:
                        key = (1, qt // 8, t)
                        col = 512 * ((qt % 8) // 4) + (qt % 4) * 128
                    return pt_tiles[key][:, col:col + 128]

                def emit_pv_chain(h, qt, t0=0, t1=NT, pv=None):
                    if pv is None:
                        if qt % 4 == 0 and t0 == 0:
                            pv_cur[0] = psv.tile(
                                [128, 4, D + 1], F32, tag="pv",
                                name=f"pv_{rep}_{h}_{qt}")
                        pv = pv_cur[0]
                    for t in range(t0, t1):
                        nc.tensor.matmul(
                            pv[:, qt % 4, :],
                            pt_col(h, qt, t),
                            v_sb[:, t, h, :],
                            start=(t == 0),
                            stop=(t == NT - 1),
                        )
                    if t1 < NT:
                        return pv
                    if qt % 4 == 3:
                        ov = opool.tile([128, 4, D + 1], F32, tag="ov",
                                        name=f"ov_{rep}_{h}_{qt}")
                        nc.vector.tensor_copy(ov, pv)
                        nc.sync.dma_start(
                            out=out_d[h, qt // 8, (qt // 4) % 2, :, :],
                            in_=ov.rearrange("p a b -> p (a b)"),
                        )

                # chains of block bi, in emission order (groups of 4)
                def block_chains(bi):
                    pair, qc = blocks[bi]
                    if pair == 0:
                        return ([(0, 4 * qc + i) for i in range(4)]
                                + [(1, 4 * qc + i) for i in range(4)])
                    return [(2, 8 * qc + i) for i in range(8)]

                # Remaining projection work rides inside the attention step
                # stream, timed to the xT column-slice DMA arrivals.
                vq = list(range(NT))
                bwork = {0: {}, 1: {}, 2: {}}
                for st in range(NT):
                    if st == 2:
                        bwork[0][st] = lambda: proj_k_chain(0, 1)
                    elif st == 6:
                        bwork[0][st] = lambda: proj_k_chain(0, 2)
                    elif st == 10:
                        bwork[0][st] = lambda: proj_k_chain(0, 3)
                    elif st == 14:
                        pass
                    else:
                        bwork[0][st] = (
                            lambda tt: (lambda: proj_v_tile(tt)))(vq.pop(0))
                for st in (0, 2, 4, 6):
                    bwork[1][st] = (
                        lambda tt: (lambda: proj_v_tile(tt)))(vq.pop(0))
                for i, st in enumerate((0, 2, 4, 6)):
                    bwork[2][st] = (
                        lambda n: (lambda: proj_k_chain(1, n)))(i)

                proj_k_chain(0, 0)

                # flat software pipeline over all (block, t) steps: scores
                # run LOOKAHEAD steps ahead of exp, across block boundaries
                steps = [(bi, t) for bi in range(len(blocks))
                         for t in range(NT)]
                n_steps = len(steps)
                final = len(blocks) - 1
                prev_ps = {}
                pvqs = {bi: block_chains(bi - 1)
                        for bi in range(1, len(blocks))}
                for i in range(n_steps + LOOKAHEAD):
                    if i < n_steps:
                        bi, t = steps[i]
                        pair, qc = blocks[bi]
                        prev_ps[(bi, t)] = emit_scores(pair, qc, t)
                        if t == 0 and bi >= 2:
                            # leftover pv chains of earlier blocks
                            for b in range(1, bi):
                                while pvqs.get(b):
                                    emit_pv_chain(*pvqs[b].pop(0))
                        if bi in bwork and t in bwork[bi]:
                            bwork[bi][t]()
                        elif t % 2 == 1 and t >= (7 if bi == 1 else 3):
                            # pv chains of the previous block on odd steps
                            # (cross-block lookahead exps land first; block
                            # 0's chains also wait for the last v-tile)
                            if pvqs.get(bi):
                                emit_pv_chain(*pvqs[bi].pop(0))
                        if bi == final and t == 14:
                            while pvqs[final]:
                                emit_pv_chain(*pvqs[final].pop(0))
                        if bi == final and t == 15:
                            # head chains of the final block's two psum banks
                            # run their first 13 accumulation steps early
                            # (one open accumulation group per bank)
                            fpv = {8: emit_pv_chain(2, 8, 0, 13),
                                   12: emit_pv_chain(2, 12, 0, 13)}
                    j = i - LOOKAHEAD
                    if j >= 0:
                        bj, tj = steps[j]
                        pj, qj = blocks[bj]
                        # the final block's last exps go to DVE so the chain
                        # drain isn't serialized behind the ACT queue
                        extra = bj == final and tj >= NT - 3
                        emit_exp(pj, qj, tj, prev_ps.pop((bj, tj)), extra)
                for qt in range(8, 16):
                    if qt in fpv:
                        emit_pv_chain(2, qt, 13, NT, pv=fpv[qt])
                    else:
                        emit_pv_chain(2, qt, pv=fpv[qt & ~3])

    nc.finalize()
    return nc


_NC = None


def _get_nc():
    global _NC
    if _NC is None:
        _NC = _build_nc()
    return _NC


def _host_prep(hidden_states, embx, expanded_embx, Wkv_w, Wkv_b,
               attention_mask, mlm_mask):
    hs = np.asarray(hidden_states, np.float32)
    qx = np.asarray(expanded_embx, np.float32)
    w = np.asarray(Wkv_w, np.float32)
    bb = np.asarray(Wkv_b, np.float32)
    am = np.asarray(attention_mask).astype(bool)
    mm = np.asarray(mlm_mask).astype(bool)

    valid = (am & ~mm).astype(np.float32)              # (B, S)

    dm = np.ones((128, 128), ml_dtypes.bfloat16)
    np.fill_diagonal(dm, 0.0)

    # per-batch tensors; xT pre-chunked as [128, KC, S]
    xT = [np.ascontiguousarray(
              hs[b].T.astype(ml_dtypes.bfloat16)
              .reshape(KC, 128, S).transpose(1, 0, 2))
          for b in range(B)]
    kbf, s1f, s2f = [], [], []
    for b in range(B):
        v = valid[b]                                   # (S,)
        kb = np.where(v > 0, 0.0, NEG).astype(np.float32)
        s1 = (v * (SA16 * SCALE)).astype(np.float32)
        s2 = (v * SB16).astype(np.float32)
        kbf.append(kb.reshape(NT, 128).T)
        s1f.append(s1.reshape(NT, 128).T)
        s2f.append(s2.reshape(NT, 128).T)

    # per-group weight layouts, pre-chunked as [128, KC, WCOLS]
    wg_l, bk_l, bv_l = [], [], []
    for g in range(4):
        k_cols = slice(192 * g, 192 * g + 192)
        v_cols = slice(768 + 192 * g, 768 + 192 * g + 192)
        wk = w[:, k_cols]                              # (768, 192)
        parts = [wk[:, 0:128],                         # [k_h0 | k_h1]
                 wk[:, 128:192], wk[:, 128:192],       # [k_h2 | k_h2]
                 w[:, v_cols]]                         # v (192)
        wg = np.concatenate(parts, axis=1).astype(ml_dtypes.bfloat16)
        wg_l.append(np.ascontiguousarray(
            wg.reshape(KC, 128, WCOLS).transpose(1, 0, 2)))
        bkk = bb[k_cols]
        bk = np.stack([bkk[0:128],
                       np.concatenate([bkk[128:192], bkk[128:192]])], axis=1)
        bk_l.append(bk.astype(np.float32))
        bv_l.append(np.broadcast_to(
            bb[v_cols], (128, G * D)).astype(np.float32))

    in_maps = []
    for c in range(NCORE):
        b, g = divmod(c, 4)
        ct = np.concatenate(
            [bk_l[g], bv_l[g], kbf[b], s1f[b], s2f[b]], axis=1)
        qg = qx[b][:, 192 * g:192 * g + 192].T         # (192, S)
        qt = np.empty((128, 2, S), ml_dtypes.bfloat16)
        qt[0:64, 0, :] = qg[0:64].astype(ml_dtypes.bfloat16)
        qt[64:128, 0, :] = qg[64:128].astype(ml_dtypes.bfloat16)
        qt[0:64, 1, :] = qg[128:192].astype(ml_dtypes.bfloat16)
        qt[64:128, 1, :] = qt[0:64, 1, :]
        in_maps.append(dict(
            xT=xT[b], W=wg_l[g], qT=np.ascontiguousarray(qt),
            ct=np.ascontiguousarray(ct), dm=dm,
        ))
    return in_maps


def _host_post(results, embx, expanded_embx, Wkv_w, Wkv_b):
    ex = np.asarray(embx, np.float32)                  # (B, 1, HID)
    qx = np.asarray(expanded_embx, np.float32)
    w = np.asarray(Wkv_w, np.float32)
    bb = np.asarray(Wkv_b, np.float32)

    # embx key: k/v projections + per-query weights, on host
    kv_eb = ex[:, 0, :] @ w + bb                       # (B, 2*HID)
    k_eb = kv_eb[:, :HID].reshape(B, H, D)
    v_eb = kv_eb[:, HID:].reshape(B, H, D)
    q3 = qx.reshape(B, S, H, D)
    s_eb = np.einsum("bshd,bhd->bsh", q3, k_eb)        # (B, S, H)
    p_eb = np.exp(SCALE * s_eb.astype(np.float64)).astype(np.float32)

    out = np.empty((B, S, HID), np.float32)
    for c in range(NCORE):
        b, g = divmod(c, 4)
        # [G, 2, 2, 128, 4, 65] -> (h, half, group, slot, row) -> (G, S, 65)
        ot = (results[c]["outT"]
              .reshape(G, 2, 2, 128, 4, D + 1)
              .transpose(0, 1, 2, 4, 3, 5)
              .reshape(G, S, D + 1))
        for h in range(G):
            hh = 3 * g + h
            num = ot[h, :, :D] + p_eb[b, :, hh:hh + 1] * v_eb[b, hh]
            den = ot[h, :, D] + p_eb[b, :, hh]
            out[b, :, 192 * g + 64 * h:192 * g + 64 * h + 64] = (
                num / den[:, None]
            )
    return out


def kernel(hidden_states, embx, expanded_embx, Wkv_w, Wkv_b,
           attention_mask, mlm_mask):
    in_maps = _host_prep(hidden_states, embx, expanded_embx, Wkv_w, Wkv_b,
                         attention_mask, mlm_mask)
    nc = _get_nc()
    res = run_bass_kernel_spmd(nc, in_maps, list(range(NCORE)))
    return _host_post(res.results, embx, expanded_embx, Wkv_w, Wkv_b)
